# revision 34
# baseline (speedup 1.0000x reference)
"""TRN2 Bass kernel for nn_Attention_23493471109551 (v3).

Full attention layer: QKV projections + interleaved RoPE + causal softmax
attention + output projection, for B=4, S=2048, D=1024, H=16, Dh=64, fp32 I/O.

Sharding: 8 cores = 4 batches x 2 head-groups (8 heads each).  Each core
computes its batch/head-group's attention and a partial output projection
(W_o row-block); host sums the two partials per batch.

v3 changes vs v2 (388us):
  - Act engine runs ONLY exp (149us): rope reads proj PSUM directly on DVE
    (no fp16 staging copy via Act), vpa copies moved to DVE, all DMA issue
    moved off the Act queue after startup.
  - normalize: reciprocal straight from the PSUM denominator row, fp16
    partition_broadcast (half the gpsimd time), multiply PSUM x broadcast
    directly into the attn tile (no posb staging copies).
  - startup: e-block-granular loads spread over sync/scalar/gpsimd queues in
    first-use order; first projection chain starts ~1.5us in.
  - emission schedule: attention (Act-heavy) is zipped at t-step granularity
    with projection chains / output projections (pure PE) so the in-order PE
    queue never drains: A0||P1, O0+A1||P2, O1+A2(hp01)||qp3+kp3, then
    A3hp_i||{A2hp23, O2, O3} pairs.  Keeps the PE p-state high.
  - fp16 partial outputs (host sums pairs in fp32): halves output DMA.

Layout (per core):
  qpT/kpT: [dh-on-partitions (2 heads x 64), hp, S] fp16
  scoresT [sk, sq] in PSUM; exp'd (scale fused) to fp16 et; PV feeds from et
  directly; denominator = ones-column appended to V (row 64 of the PV psum);
  normalization = approx-reciprocal + fp16 gpsimd partition_broadcast + mul.
"""
import math
import numpy as np

import concourse.bass as bass
import concourse.tile as tile
import concourse.mybir as mybir
from concourse import bacc, bass_utils

# problem constants
B, S, D = 4, 2048, 1024
H, Dh = 16, 64
EQ, EV = 2048, 1024          # q/k and v input feature dims
F = 512                      # features per core (8 heads x 64)
P = 128
N_CORES = 8
SCALE = 1.0 / math.sqrt(D)   # 1/32
ROPE_BASE = 10000.0
SWAP_MASK = [i ^ 1 for i in range(32)]

F16 = mybir.dt.float16
F32 = mybir.dt.float32

# test hooks (harness ignores these)
KERNEL_TRACE = False
LAST_RESULT = None

_nc_cache = None


def _chain(*gens):
    for g in gens:
        yield from g


def _take(g, n):
    for _ in range(n):
        yield next(g)


def _emit_zip(*weighted):
    """Round-robin emission of (generator, steps_per_round) pairs."""
    live = [[iter(g), w] for g, w in weighted]
    while live:
        for entry in live[:]:
            g, w = entry
            for _ in range(w):
                try:
                    next(g)
                except StopIteration:
                    live.remove(entry)
                    break


def _build_nc():
    # x tensors arrive host-relayouted as [c, p, e, s_c] and weights as
    # [p, e, f]: every DMA then moves >=4KB contiguous per partition (vs 1KB
    # per (p, e) row in the naive [f, s] layout).  The input feed was
    # descriptor-rate-bound (~190ns per 1KB descriptor), not bandwidth-bound.
    nc = bacc.Bacc("TRN2", target_bir_lowering=False, debug=False)
    qT = nc.dram_tensor("qT", [4 * P * 16, 512], F16, kind="ExternalInput").ap()
    kT = nc.dram_tensor("kT", [4 * P * 16, 512], F16, kind="ExternalInput").ap()
    vT = nc.dram_tensor("vT", [4 * P * 8, 512], F16, kind="ExternalInput").ap()
    wqT = nc.dram_tensor("wqT", [P, 16 * F], F16, kind="ExternalInput").ap()
    wkT = nc.dram_tensor("wkT", [P, 16 * F], F16, kind="ExternalInput").ap()
    wvT = nc.dram_tensor("wvT", [P, 8 * F], F16, kind="ExternalInput").ap()
    woT = nc.dram_tensor("woT", [P, 4 * D], F16, kind="ExternalInput").ap()
    cosf = nc.dram_tensor("cosf", [P, S], F16, kind="ExternalInput").ap()
    sinf = nc.dram_tensor("sinf", [P, S], F16, kind="ExternalInput").ap()
    maskA = nc.dram_tensor("maskA", [P, P], F16, kind="ExternalInput").ap()
    out = nc.dram_tensor("out", [S, D], F16, kind="ExternalOutput").ap()

    EXP = mybir.ActivationFunctionType.Exp

    with tile.TileContext(nc) as tc:
        with (
            tc.tile_pool(name="consts", bufs=1) as consts,
            tc.tile_pool(name="persist", bufs=1) as persist,
            tc.tile_pool(name="insb", bufs=2) as insb_pool,
            tc.tile_pool(name="vsb", bufs=2) as vsb_pool,
            tc.tile_pool(name="rope", bufs=2) as rope_pool,
            tc.tile_pool(name="et", bufs=4) as et_pool,
            tc.tile_pool(name="norm", bufs=2) as norm_pool,
            tc.tile_pool(name="attnc", bufs=2) as attnc_pool,
            tc.tile_pool(name="outsb", bufs=2) as out_pool,
            tc.tile_pool(name="finout", bufs=1) as finout_pool,
            tc.tile_pool(name="pops", bufs=2, space="PSUM") as po_ps,
        ):
            # score-psum pool is released before the final outproj so its 4
            # banks (plus the tail pool's 2) can hold chunk-3's pw tiles
            sc_ps = tc.alloc_tile_pool(name="scps", bufs=2, space="PSUM")
            # released after the last projection so the tail (chunk-3
            # attention) gets its 2 banks for a second po accumulator set
            proj_ps = tc.alloc_tile_pool(name="projps", bufs=2, space="PSUM")
            # ---- persistent activations
            qpT = persist.tile([P, 4, S], F16, tag="qpT")
            kpT = persist.tile([P, 4, S], F16, tag="kpT")
            vpa = persist.tile([P, 16, 8, 65], F16, tag="vpa")
            nc.vector.memset(vpa[:, :, :, 64:65], 1.0)  # softmax-denominator ones

            # ---- weights + tables
            wq_t = consts.tile([P, 16, F], F16, tag="wq")
            wk_t = consts.tile([P, 16, F], F16, tag="wk")
            wv_t = consts.tile([P, 8, F], F16, tag="wv")
            wo_t = consts.tile([P, 4, D], F16, tag="wo")
            cos_t = consts.tile([P, S], F16, tag="cos")
            sin_t = consts.tile([P, S], F16, tag="sin")
            mask_t = consts.tile([P, P], F16, tag="mask")
            # views of the HBM inputs: x as [c, p, e, s_c], weights [p, e, f]
            qTr = qT.rearrange("(c p e) s -> c p e s", p=P, e=16)
            kTr = kT.rearrange("(c p e) s -> c p e s", p=P, e=16)
            vTr = vT.rearrange("(c p e) s -> c p e s", p=P, e=8)
            wqTr = wqT.rearrange("p (e f) -> p e f", e=16)
            wkTr = wkT.rearrange("p (e f) -> p e f", e=16)
            wvTr = wvT.rearrange("p (e f) -> p e f", e=8)
            woTr = woT.rearrange("p (e f) -> p e f", e=4)

            def bulk(dmae, dst, src, n_e, split):
                step = n_e // split
                for i in range(0, n_e, step):
                    dmae.dma_start(dst[:, i:i + step], src[:, i:i + step])

            # ---- startup loads, e-block granular, ordered by first use.
            # gpsimd (cheap issue): wq e-pairs interleaved with c0 rope
            # tables, then wv + vsb0.  sync: qsb0 pieces, mask, wo.
            # scalar (startup only -- Act must stay exp-only later):
            # ksb0 + wk pieces.
            qsb0 = insb_pool.tile([P, 16, 512], F16, tag="insb", name="qsb0")
            ksb0 = insb_pool.tile([P, 16, 512], F16, tag="insb", name="ksb0")
            vsb0 = vsb_pool.tile([P, 8, 512], F16, tag="vsb", name="vsb0")
            pieces = [(0, 2), (2, 4), (4, 8), (8, 12), (12, 16)]
            for a, b in pieces:
                nc.sync.dma_start(qsb0[:, a:b], qTr[0, :, a:b, :])
            nc.gpsimd.dma_start(wq_t[:, 0:2], wqTr[:, 0:2])
            nc.gpsimd.dma_start(cos_t[:, 0:512], cosf[:, 0:512])
            nc.gpsimd.dma_start(sin_t[:, 0:512], sinf[:, 0:512])
            for a, b in pieces[1:]:
                nc.gpsimd.dma_start(wq_t[:, a:b], wqTr[:, a:b])
            # interleave ksb0/wk e-pieces: the k chains consume both in
            # e-order, so neither may trail the other by a full tensor
            for a, b in pieces:
                nc.scalar.dma_start(ksb0[:, a:b], kTr[0, :, a:b, :])
                nc.scalar.dma_start(wk_t[:, a:b], wkTr[:, a:b])
            nc.gpsimd.dma_start(wv_t[:], wvTr[:])
            nc.gpsimd.dma_start(vsb0[:], vTr[0])
            for sl in (slice(512, 1024), slice(1024, 2048)):
                nc.gpsimd.dma_start(cos_t[:, sl], cosf[:, sl])
                nc.gpsimd.dma_start(sin_t[:, sl], sinf[:, sl])

            def load_qk(c):
                qsb = insb_pool.tile([P, 16, 512], F16, tag="insb", name=f"qsb{c}")
                for a, b in ((0, 8), (8, 16)):
                    nc.sync.dma_start(qsb[:, a:b], qTr[c, :, a:b, :])
                ksb = insb_pool.tile([P, 16, 512], F16, tag="insb", name=f"ksb{c}")
                for a, b in ((0, 8), (8, 16)):
                    nc.gpsimd.dma_start(ksb[:, a:b], kTr[c, :, a:b, :])
                vsb = vsb_pool.tile([P, 8, 512], F16, tag="vsb", name=f"vsb{c}")
                nc.gpsimd.dma_start(vsb[:], vTr[c])
                return qsb, ksb, vsb

            def proj_x(c, xsb, w_t, dstT, nm):
                """One tensor's projection for s-chunk c + rope; yields per
                ci-chain (16 chained matmuls + rope on DVE)."""
                ssl = slice(c * 512, (c + 1) * 512)
                for ci in range(4):
                    ps = proj_ps.tile([P, 512], F32, tag="proj",
                                      name=f"ps{nm}{c}_{ci}")
                    for e in range(16):
                        nc.tensor.matmul(ps[:], w_t[:, e, ci * P:(ci + 1) * P],
                                         xsb[:, e, :],
                                         start=(e == 0), stop=(e == 15))
                    # rope: out = x*cos + pairswap(x*sin'), x read from PSUM
                    cm = rope_pool.tile([P, 512], F16, tag="ropeC")
                    nc.vector.tensor_mul(cm[:], ps[:], cos_t[:, ssl])
                    sm = rope_pool.tile([P, 512], F16, tag="ropeS")
                    nc.vector.tensor_mul(sm[:], ps[:], sin_t[:, ssl])
                    sm2 = rope_pool.tile([P, 512], F16, tag="ropeS2")
                    nc.vector.stream_shuffle(sm2[:], sm[:], SWAP_MASK)
                    nc.vector.tensor_add(dstT[:, ci, ssl], cm[:], sm2[:])
                    yield

            def proj_qk(c, qsb, ksb):
                yield from proj_x(c, qsb, wq_t, qpT, "q")
                yield from proj_x(c, ksb, wk_t, kpT, "k")

            def proj_v(stq, vsb):
                """Project v for s-chunk stq into vpa (s on partitions)."""
                for j in range(4):
                    ps = proj_ps.tile([P, 512], F32, tag="proj",
                                      name=f"psv{stq}_{j}")
                    for e in range(8):
                        nc.tensor.matmul(ps[:], vsb[:, e, j * P:(j + 1) * P],
                                         wv_t[:, e, :],
                                         start=(e == 0), stop=(e == 7))
                    st = stq * 4 + j
                    nc.vector.tensor_copy(
                        vpa[:, st, :, 0:64],
                        ps[:].rearrange("p (h d) -> p h d", h=8))
                    yield

            def attn_stream(c, hp, attn_c, po_pool, main_pool=True):
                """Causal attention for query chunk c, head-pair hp; yields
                after each key-tile step and after the normalize."""
                nt = 4 * (c + 1)
                if main_pool:
                    po_a = po_pool.tile([P, 512], F32, tag="po",
                                        name=f"poa{c}_{hp}")
                    po_b = po_pool.tile([P, 512], F32, tag="po",
                                        name=f"pob{c}_{hp}")
                else:
                    po_a = po_pool.tile([P, 512], F32, tag="poa",
                                        name=f"tpoa{c}_{hp}")
                    po_b = po_pool.tile([P, 512], F32, tag="pob",
                                        name=f"tpob{c}_{hp}")
                for t in range(nt):
                    tsl = slice(t * P, (t + 1) * P)
                    rr = P * (t - 4 * c) if t >= 4 * c else 0
                    qsl = slice(c * 512 + rr, (c + 1) * 512)
                    ps_s = sc_ps.tile([P, 2, 512], F32, tag="sc",
                                      name=f"scs{c}_{hp}_{t}")
                    nc.tensor.matmul(ps_s[:, 0, rr:512], kpT[0:64, hp, tsl],
                                     qpT[0:64, hp, qsl], start=True, stop=True)
                    nc.tensor.matmul(ps_s[:, 1, rr:512], kpT[64:128, hp, tsl],
                                     qpT[64:128, hp, qsl], start=True, stop=True)
                    et = et_pool.tile([P, 2, 512], F16, tag="et")
                    nc.scalar.activation(et[:, :, rr:512], ps_s[:, :, rr:512],
                                         EXP, scale=SCALE)
                    if t >= 4 * c:
                        # zero the above-diagonal triangle of this block
                        nc.vector.tensor_mul(
                            et[:, :, rr:rr + P], et[:, :, rr:rr + P],
                            mask_t[:, None, :].to_broadcast((P, 2, P)))
                    nc.tensor.matmul(po_a[0:65, rr:512], vpa[:, t, 2 * hp, :],
                                     et[:, 0, rr:512],
                                     start=(t == 0), stop=(t == nt - 1))
                    nc.tensor.matmul(po_b[0:65, rr:512], vpa[:, t, 2 * hp + 1, :],
                                     et[:, 1, rr:512],
                                     start=(t == 0), stop=(t == nt - 1))
                    yield
                # normalize: attn = po[0:64] * (1/po[64]) per head.  den is
                # staged to base partition 0 (cross-base DVE inputs
                # miscompile), reciprocal'd, cast fp16, broadcast on gpsimd,
                # then multiplied against the PSUM rows directly.  Emission
                # yields between op groups so zipped partners (proj-chain
                # rope) aren't queued behind the whole 6-op DVE burst.
                for po, half in ((po_a, slice(0, 64)), (po_b, slice(64, 128))):
                    den = norm_pool.tile([1, 512], F32, tag="den")
                    nc.vector.tensor_copy(den[:], po[64:65, :])
                    rc = norm_pool.tile([1, 512], F32, tag="rc")
                    nc.vector.reciprocal_approx_fast(out=rc[:], in_=den[:])
                    rc16 = norm_pool.tile([1, 512], F16, tag="rc16")
                    nc.vector.tensor_copy(rc16[:], rc[:])
                    yield
                    bc = norm_pool.tile([64, 512], F16, tag="bc")
                    nc.gpsimd.partition_broadcast(bc[:], rc16[:])
                    nc.vector.tensor_mul(attn_c[half, hp, :], po[0:64, :], bc[:])
                    yield

            def outproj(c, attn_c):
                """Output projection for chunk c; yields per q-block j."""
                for j in range(4):
                    pw = [po_ps.tile([P, 512], F32, tag="po", name=f"pw{c}_{j}_{i}")
                          for i in range(2)]
                    for ci in range(4):
                        for oc in range(2):
                            nc.tensor.matmul(pw[oc][:],
                                             attn_c[:, ci, j * P:(j + 1) * P],
                                             wo_t[:, ci, oc * 512:(oc + 1) * 512],
                                             start=(ci == 0), stop=(ci == 3))
                    row = (4 * c + j) * P
                    for oc in range(2):
                        ot = out_pool.tile([P, 512], F16, tag="ot")
                        nc.vector.tensor_copy(ot[:], pw[oc][:])
                        dmae = nc.sync if oc == 0 else nc.gpsimd
                        dmae.dma_start(out[row:row + P, oc * 512:(oc + 1) * 512],
                                       ot[:])
                    yield

            def outproj_final(c, attn_c, final_ps):
                """Chunk-3 output projection: q-blocks j0-j2 run ci-major on
                6 early-freed PSUM banks (ex-score + po), so only the ci=3
                pass waits on the final head-pair's normalize; j3 follows.
                Copies split across the (idle) Act engine and DVE; DMA
                issues split across sync/gpsimd."""
                dmaq = [nc.sync, nc.gpsimd]

                def emit_copy(j, oc, pwt, n):
                    row = (4 * c + j) * P
                    ot = finout_pool.tile([P, 512], F16, tag=f"otf{j}_{oc}",
                                          name=f"otf{j}_{oc}")
                    if n % 2:
                        nc.scalar.copy(ot[:], pwt[:])
                    else:
                        nc.vector.tensor_copy(ot[:], pwt[:])
                    dmaq[oc].dma_start(
                        out[row:row + P, oc * 512:(oc + 1) * 512], ot[:])

                pw = {}
                for j in range(3):
                    pool = final_ps if j < 2 else po_ps
                    tag = f"fin{j}" if j < 2 else "po"
                    for oc in range(2):
                        pw[(j, oc)] = pool.tile([P, 512], F32, tag=tag,
                                                name=f"pwf{j}_{oc}")
                for ci in range(4):
                    for j in range(3):
                        for oc in range(2):
                            nc.tensor.matmul(pw[(j, oc)][:],
                                             attn_c[:, ci, j * P:(j + 1) * P],
                                             wo_t[:, ci, oc * 512:(oc + 1) * 512],
                                             start=(ci == 0), stop=(ci == 3))
                for n, (j, oc) in enumerate(sorted(pw)):
                    emit_copy(j, oc, pw[(j, oc)], n)
                pw3 = [po_ps.tile([P, 512], F32, tag="po", name=f"pwf3_{i}")
                       for i in range(2)]
                for ci in range(4):
                    for oc in range(2):
                        nc.tensor.matmul(pw3[oc][:],
                                         attn_c[:, ci, 3 * P:4 * P],
                                         wo_t[:, ci, oc * 512:(oc + 1) * 512],
                                         start=(ci == 0), stop=(ci == 3))
                for oc in range(2):
                    emit_copy(3, oc, pw3[oc], oc)

            def drain(g):
                for _ in g:
                    pass

            def attn_seq(c, attn_c, hps, pool=po_ps, main_pool=True):
                for hp in hps:
                    yield from attn_stream(c, hp, attn_c, pool, main_pool)

            # ---- program ----
            attn_t = [attnc_pool.tile([P, 4, 512], F16, tag="attn",
                                      name=f"attn{c}") for c in range(4)]

            # chunk 0 projections (sequential; nothing else to overlap)
            drain(proj_qk(0, qsb0, ksb0))
            drain(proj_v(0, vsb0))

            # A0 zipped with proj1.  mask/wo loads issue AFTER the chunk-1
            # input loads so qsb1/ksb1 aren't queued behind wo's 1MB.
            qsb1, ksb1, vsb1 = load_qk(1)
            nc.sync.dma_start(mask_t[:], maskA)
            bulk(nc.sync, wo_t, woTr, 4, 2)
            _emit_zip((attn_seq(0, attn_t[0], range(4)), 2),
                      (_chain(proj_qk(1, qsb1, ksb1), proj_v(1, vsb1)), 1))

            # O0 + A1 zipped with proj2
            qsb2, ksb2, vsb2 = load_qk(2)
            _emit_zip((_chain(outproj(0, attn_t[0]),
                              attn_seq(1, attn_t[1], range(4))), 2),
                      (_chain(proj_qk(2, qsb2, ksb2), proj_v(2, vsb2)), 1))

            # O1 + A2(hp0,hp1) zipped with qp3+kp3
            qsb3, ksb3, vsb3 = load_qk(3)
            _emit_zip((_chain(outproj(1, attn_t[1]),
                              attn_seq(2, attn_t[2], (0, 1))), 2),
                      (proj_qk(3, qsb3, ksb3), 1))
            drain(proj_v(3, vsb3))
            proj_ps.release()

            # tail: A3 head-pairs (tail psum banks) zipped with the
            # remaining A2 head-pairs / O2 (main po banks).  outproj(3) must
            # be emitted strictly AFTER every A3 normalize: a reader emitted
            # before its writer exists gets no dependency edge (reads stale
            # data), so O3 cannot be zipped with the last A3 streams.
            tail_ps = tc.alloc_tile_pool(name="tailps", bufs=1, space="PSUM")
            _emit_zip((attn_stream(3, 0, attn_t[3], tail_ps, False), 1),
                      (attn_stream(2, 2, attn_t[2], po_ps), 1))
            _emit_zip((attn_stream(3, 1, attn_t[3], tail_ps, False), 1),
                      (attn_stream(2, 3, attn_t[2], po_ps), 1))
            o2 = outproj(2, attn_t[2])
            _emit_zip((attn_stream(3, 2, attn_t[3], tail_ps, False), 6),
                      (_take(o2, 2), 1))
            _emit_zip((attn_stream(3, 3, attn_t[3], tail_ps, False), 6),
                      (o2, 1))
            # LIFO pool release; the 4-bank finale pool lands on the ex-score
            # banks (lowest offsets, gated only by the last exp reads) -- the
            # ex-tail banks, gated on the LAST normalize, stay unused
            tail_ps.release()
            sc_ps.release()
            final_ps = tc.alloc_tile_pool(name="finps", bufs=1, space="PSUM")
            outproj_final(3, attn_t[3], final_ps)
            final_ps.release()
    nc.compile()
    return nc


def _tables():
    inv = (1.0 / (ROPE_BASE ** (np.arange(0, Dh, 2, dtype=np.float32) / Dh))
           ).astype(np.float32)                      # [32]
    pos = np.arange(S, dtype=np.float32)
    ang = pos[:, None] * inv[None, :]                # [S, 32]
    cos = np.cos(ang).astype(np.float32)
    sin = np.sin(ang).astype(np.float32)
    d = np.arange(P) % Dh
    i = d // 2
    cosf = np.ascontiguousarray(cos[:, i].T).astype(np.float16)   # [128, S]
    sgn = np.where(d % 2 == 0, 1.0, -1.0).astype(np.float32)
    sinf = np.ascontiguousarray(sin[:, i].T * sgn[:, None]).astype(np.float16)

    p = np.arange(P)
    j = np.arange(P)
    maskA = np.where(p[:, None] <= j[None, :], 1.0, 0.0).astype(np.float16)
    return cosf, sinf, maskA


def kernel(q, k, v, W_q, W_k, W_v, W_o):
    global _nc_cache, LAST_RESULT
    if _nc_cache is None:
        _nc_cache = _build_nc()
    nc = _nc_cache

    cosf, sinf, maskA = _tables()
    q = np.asarray(q, dtype=np.float32)
    k = np.asarray(k, dtype=np.float32)
    v = np.asarray(v, dtype=np.float32)
    W_q = np.asarray(W_q, dtype=np.float32)
    W_k = np.asarray(W_k, dtype=np.float32)
    W_v = np.asarray(W_v, dtype=np.float32)
    W_o = np.asarray(W_o, dtype=np.float32)

    def xlay(xT, e):
        # [e*128, 2048] -> [c, p, e, s_c] flat: per-partition-contiguous DMA
        return np.ascontiguousarray(
            xT.astype(np.float16).reshape(e, P, 4, 512).transpose(2, 1, 0, 3)
        ).reshape(4 * P * e, 512)

    def wlay(wT, e):
        # [e*128, f] -> [p, e*f]: per-partition-contiguous DMA
        f = wT.shape[1]
        return np.ascontiguousarray(
            wT.astype(np.float16).reshape(e, P, f).transpose(1, 0, 2)
        ).reshape(P, e * f)

    in_maps = []
    for b in range(B):
        qTb = xlay(q[b].T, 16)
        kTb = xlay(k[b].T, 16)
        vTb = xlay(v[b].T, 8)
        for g in range(2):
            fs = slice(g * F, (g + 1) * F)
            in_maps.append({
                "qT": qTb, "kT": kTb, "vT": vTb,
                "wqT": wlay(W_q[fs, :].T, 16),
                "wkT": wlay(W_k[fs, :].T, 16),
                "wvT": wlay(W_v[fs, :].T, 8),
                "woT": wlay(W_o[:, fs].T, 4),
                "cosf": cosf, "sinf": sinf, "maskA": maskA,
            })

    res = bass_utils.run_bass_kernel_spmd(
        nc, in_maps, core_ids=list(range(N_CORES)), trace=KERNEL_TRACE)
    LAST_RESULT = res

    final = np.empty((B, S, D), dtype=np.float32)
    for b in range(B):
        final[b] = (res.results[2 * b]["out"].astype(np.float32)
                    + res.results[2 * b + 1]["out"].astype(np.float32))
    return final
